# revision 11
# baseline (speedup 1.0000x reference)
"""KernelConv2D (per-pixel dynamic 5x5 depthwise conv) on 8 TRN2 NeuronCores.

Problem: out[b,c,h,w] = sum_{i,j} x_edgepad[b,c,h+i,w+j] * K[b,c,i,j,h,w]
with input [4,32,128,128] f32 and kernel [4,800,128,128] f32 (800 = 32*25).

Sharding: every (b,c) plane is independent -> flatten to 128 planes on the
SBUF partition axis; each core takes 16 output rows of all 128 planes.

The problem is HBM-bound on reading K (210 MB f32). The harness gate is
rel_l2 < 2e-2, so K, x and the output travel as bf16 (~2e-3 error), halving
traffic to ~15 MB/core ~= 42 us at the ~358 GB/s per-NC HBM limit.

bf16 also doubles DVE throughput (2x_1P mode), but that mode needs step-1
4B-aligned streams, and the 5 horizontal taps read x at column offsets
0..4 (alternating 2-byte alignment). Fix: keep 5 column-shifted copies of
the padded x band in SBUF (planes 0/1 from host, planes 2/3/4 derived by
the otherwise-idle ScalarE with aligned +2/+4 shifts). A product op then
covers one vertical tap i x all 5 j in a single 3-free-dim AP at 2x.

Reduction of the 25 bf16 product segments runs on the TensorEngine as
1-pass identity matmuls accumulating into one PSUM bank (f32 adds).
ScalarE evacuates PSUM -> SBUF with an f32->bf16 cast; stores are bf16.

Per-core budget: DMA 15.0 MB ~= 42 us (wall), DVE ~31 us, PE ~17-22 us,
ACT ~6 us. Chunks of [4,4,4,3,1] rows keep K loads >=1 MB (near line
rate) and shrink the compute tail after the last K byte lands.
"""

import sys

import numpy as np

sys.path.insert(0, "/opt/trn_rl_repo")

import ml_dtypes

import concourse.bacc as bacc
import concourse.bass as bass
import concourse.tile as tile
from concourse import mybir
from concourse.ap import AP
from concourse.bass_utils import run_bass_kernel_spmd

N_CORES = 8
B, C, H, W, KS = 4, 32, 128, 128, 5
NPLANES = B * C          # 128 -> partition axis
NTAPS = KS * KS          # 25
ROWS_PER_CORE = H // N_CORES   # 16
CHUNK_ROWS = [4, 4, 4, 3, 1]
CHUNK_STARTS = [0, 4, 8, 12, 15]
NCHUNK = len(CHUNK_ROWS)
RMAX = max(CHUNK_ROWS)
FDW = RMAX * W                             # max output elems per chunk-partition
XW = W + KS - 1                            # 132 padded row width
XROWS = ROWS_PER_CORE + KS - 1             # 20 rows incl halo
XPLANE = XROWS * XW                        # 2640 elems per shifted x copy
F32 = mybir.dt.float32
BF16 = mybir.dt.bfloat16
BFNP = ml_dtypes.bfloat16

_compiled = None


def _build_program():
    nc = bacc.Bacc(
        "TRN2",
        target_bir_lowering=False,
        debug=False,
        enable_asserts=False,
        num_devices=N_CORES,
    )
    # Host pre-arranges k as [plane][chunk][tap][h][w] so each chunk load is
    # one contiguous per-partition run (few DMA descriptors, near line rate).
    xd = nc.declare_dram_parameter("x", [NPLANES, XPLANE], BF16, isOutput=False)
    kd = nc.declare_dram_parameter(
        "k", [NPLANES, NTAPS * ROWS_PER_CORE * W], BF16, isOutput=False
    )
    od = nc.declare_dram_parameter("o", [NPLANES, ROWS_PER_CORE * W], BF16, isOutput=True)
    ed = nc.declare_dram_parameter("eye", [NPLANES, NPLANES], BF16, isOutput=False)

    with tile.TileContext(nc) as tc:
        with (
            tc.tile_pool(name="xpool", bufs=1) as xpool,
            tc.tile_pool(name="epool", bufs=1) as epool,
            tc.tile_pool(name="kpool", bufs=4) as kpool,
            tc.tile_pool(name="ppool", bufs=2) as ppool,
            tc.tile_pool(name="spool", bufs=2, space="PSUM") as spool,
            tc.tile_pool(name="opool", bufs=2) as opool,
        ):
            # The tiny eye load (256 B/partition, descriptor-dominated) rides
            # the otherwise-empty ACT ring; x goes FIRST on the sync ring so
            # it is never starved behind the 13 MB K stream (measured 4.5:1).
            et = epool.tile([NPLANES, NPLANES], BF16)
            nc.scalar.dma_start(out=et[:], in_=ed.ap())

            # 5 column-shifted copies of the padded x band: plane j holds
            # xpad[r][c+j]. Planes 0/1 come from HBM (the odd shift cannot be
            # derived on-device at full speed: a +1 source is 2B-misaligned);
            # planes 2/3/4 are +2/+2/+4 shifted flat copies on DVE (aligned
            # even-length 1-D bf16 copies -> 4x mode, ~0.75us each). Copy
            # lengths are trimmed 2-4 elems so each gates on only one x DMA;
            # the missing tail cols (>=128) are never read by the products.
            xt = xpool.tile([NPLANES, 5 * XPLANE], BF16)
            nc.sync.dma_start(out=xt[:, 0:XPLANE], in_=xd.ap()[:, 0:XPLANE])
            # Planes 1-4 are +1..+4 column-shifted flat copies of plane 0,
            # made by the otherwise-idle ScalarE (1 elem/cycle regardless of
            # alignment). Copy lengths are trimmed so nothing reads past the
            # band; the missing tail cols (>=128) are never read. Plane 1
            # gates chunk 0's odd-parity products (~12us); planes 2-4 only
            # gate chunk 1 (~21us).
            for sh in (1, 2, 3, 4):
                nc.scalar.copy(
                    xt[:, sh * XPLANE : (sh + 1) * XPLANE - sh], xt[:, sh:XPLANE]
                )
            xt_ap = xt[:]
            xt_pdim = xt_ap.ap[0]  # (partition step, 128)

            for ch in range(NCHUNK):
                h0 = CHUNK_STARTS[ch]
                rows = CHUNK_ROWS[ch]
                rw = rows * W
                kt = kpool.tile([NPLANES, NTAPS * FDW], BF16, tag="kt")
                base = NTAPS * W * h0
                seg = KS * rw
                # Chunk 0 loads per i-group (5 taps), interleaved with the x
                # planes, so the first products start as early as possible;
                # later chunks use two sub-loads (products for taps 0-9 gate
                # on the first half only). The 1-row tail chunk loads whole.
                # Chunk 0: per-i-group loads (taps host-reordered [0,2,4,1,3]
                # within each i), with i=0 further split 3+2 so the first
                # even-parity product gates on just 0.39 MB. Later chunks use
                # a 10/15-tap split; products for taps 0-9 gate on the first
                # half only. Too many small DMAs serialize on trigger
                # sem-lane reuse, so granularity stays coarse mid-stream.
                if ch == 0:
                    nc.sync.dma_start(
                        out=kt[:, 0 : 3 * rw], in_=kd.ap()[:, base : base + 3 * rw]
                    )
                    nc.sync.dma_start(
                        out=kt[:, 3 * rw : seg],
                        in_=kd.ap()[:, base + 3 * rw : base + seg],
                    )
                    for i in range(1, KS):
                        nc.sync.dma_start(
                            out=kt[:, i * seg : (i + 1) * seg],
                            in_=kd.ap()[:, base + i * seg : base + (i + 1) * seg],
                        )
                else:
                    nc.sync.dma_start(
                        out=kt[:, 0 : 10 * rw],
                        in_=kd.ap()[:, base : base + 10 * rw],
                    )
                    nc.sync.dma_start(
                        out=kt[:, 10 * rw : NTAPS * rw],
                        in_=kd.ap()[:, base + 10 * rw : base + NTAPS * rw],
                    )

                pt = ppool.tile([NPLANES, NTAPS * FDW], BF16, tag="pt")
                st = spool.tile([NPLANES, FDW], F32, tag="st")
                for i in range(KS):
                    if ch == 0:
                        # Parity-split products: even j from plane 0, odd j
                        # from plane 1, j-stride 2 elems (4B) keeps 2x_1P
                        # alignment without waiting for the shifted copies.
                        kt_ap = kt[:]
                        pt_ap = pt[:]
                        for par, nj in ((0, 3), (1, 2)):
                            p0 = i * KS + (0 if par == 0 else 3)
                            k_view = AP(
                                kt_ap.tensor,
                                kt_ap.offset + p0 * rw,
                                [kt_ap.ap[0], (rw, nj), (W, rows), (1, W)],
                            )
                            p_view = AP(
                                pt_ap.tensor,
                                pt_ap.offset + p0 * rw,
                                [pt_ap.ap[0], (rw, nj), (W, rows), (1, W)],
                            )
                            x_view = AP(
                                xt_ap.tensor,
                                xt_ap.offset + par * XPLANE + (h0 + i) * XW,
                                [xt_pdim, (2, nj), (XW, rows), (1, W)],
                            )
                            nc.vector.tensor_mul(p_view, k_view, x_view)
                            for q in range(nj):
                                pos = p0 + q
                                nc.tensor.matmul(
                                    st[:, 0:rw],
                                    et[:],
                                    pt[:, pos * rw : (pos + 1) * rw],
                                    start=(pos == 0),
                                    stop=(pos == NTAPS - 1),
                                )
                        continue
                    # One DVE op per vertical tap i covers the 5 horizontal
                    # taps j as the shifted-plane axis (stride XPLANE): 3 free
                    # dims, all strides even, rows step-1 -> bf16 2x_1P mode.
                    # TensorE folds each i-group's segments into PSUM as soon
                    # as the product lands.
                    k_view = kt[:, i * seg : (i + 1) * seg].rearrange(
                        "p (j h w) -> p j h w", j=KS, h=rows, w=W
                    )
                    p_view = pt[:, i * seg : (i + 1) * seg].rearrange(
                        "p (j h w) -> p j h w", j=KS, h=rows, w=W
                    )
                    x_view = AP(
                        xt_ap.tensor,
                        xt_ap.offset + (h0 + i) * XW,
                        [xt_pdim, (XPLANE, KS), (XW, rows), (1, W)],
                    )
                    nc.vector.tensor_mul(p_view, k_view, x_view)
                    for j in range(KS):
                        t = i * KS + j
                        nc.tensor.matmul(
                            st[:, 0:rw],
                            et[:],
                            pt[:, t * rw : (t + 1) * rw],
                            start=(t == 0),
                            stop=(t == NTAPS - 1),
                        )

                # ScalarE: evacuate PSUM -> SBUF with f32->bf16 cast, store.
                ot = opool.tile([NPLANES, FDW], BF16, tag="ot")
                nc.scalar.copy(ot[:, 0:rw], st[:, 0:rw])
                nc.scalar.dma_start(
                    out=od.ap()[:, h0 * W : h0 * W + rw], in_=ot[:, 0:rw]
                )

    nc.compile()
    return nc


def _get_program():
    global _compiled
    if _compiled is None:
        _compiled = _build_program()
    return _compiled


def _shard_inputs(input: np.ndarray, kernel: np.ndarray):
    x = np.ascontiguousarray(input, dtype=np.float32).reshape(NPLANES, H, W)
    # Edge padding: 2 each side for the conv, +1 extra right col so the
    # odd-shifted copy can take a full 132-wide slice.
    xp = np.pad(x, ((0, 0), (2, 2), (2, 3)), mode="edge").astype(BFNP)  # [128,132,133]
    k = np.ascontiguousarray(kernel, dtype=np.float32).reshape(
        NPLANES, NTAPS, H, W
    )
    eye = np.eye(NPLANES, dtype=BFNP)
    in_maps = []
    for c in range(N_CORES):
        r0 = c * ROWS_PER_CORE
        xb = xp[:, r0 : r0 + XROWS, :]  # [128, 20, 133]
        x2 = np.ascontiguousarray(xb[:, :, 0:XW]).reshape(NPLANES, XPLANE)
        ks = k[:, :, r0 : r0 + ROWS_PER_CORE, :]
        # Chunk 0's taps are reordered [0,2,4,1,3] within each i-group so the
        # even-parity (plane-0) products read contiguous segments and the
        # first product gates on a minimal leading load.
        perm0 = [i * KS + j for i in range(KS) for j in (0, 2, 4, 1, 3)]
        blocks = [
            ks[:, perm0 if ci == 0 else slice(None), s : s + n, :].reshape(
                NPLANES, NTAPS * n * W
            )
            for ci, (s, n) in enumerate(zip(CHUNK_STARTS, CHUNK_ROWS))
        ]
        kc = np.ascontiguousarray(np.concatenate(blocks, axis=1)).astype(BFNP)
        in_maps.append(
            {
                "x": np.ascontiguousarray(x2),
                "k": kc,
                "eye": eye,
            }
        )
    return in_maps


last_results = None  # BassKernelResults of the most recent run (for profiling)


def kernel(input: np.ndarray, kernel: np.ndarray, _trace: bool = False):
    global last_results
    nc = _get_program()
    in_maps = _shard_inputs(input, kernel)
    res = run_bass_kernel_spmd(nc, in_maps, list(range(N_CORES)), trace=_trace)
    last_results = res
    out = np.empty((NPLANES, H, W), dtype=np.float32)
    for c in range(N_CORES):
        out[:, c * ROWS_PER_CORE : (c + 1) * ROWS_PER_CORE, :] = (
            np.asarray(res.results[c]["o"])
            .astype(np.float32)
            .reshape(NPLANES, ROWS_PER_CORE, W)
        )
    return out.reshape(B, C, H, W)


if __name__ == "__main__":
    rng = np.random.default_rng(0)
    inp = rng.standard_normal((B, C, H, W), dtype=np.float32)
    kern = rng.standard_normal((B, C * NTAPS, H, W), dtype=np.float32)
    out = kernel(inp, kern)
    print("ran ok", out.shape, out.dtype)


# revision 12
# speedup vs baseline: 1.1992x; 1.1992x over previous
"""KernelConv2D (per-pixel dynamic 5x5 depthwise conv) on 8 TRN2 NeuronCores.

Problem: out[b,c,h,w] = sum_{i,j} x_edgepad[b,c,h+i,w+j] * K[b,c,i,j,h,w]
with input [4,32,128,128] f32 and kernel [4,800,128,128] f32 (800 = 32*25).

Sharding: every (b,c) plane is independent -> flatten to 128 planes on the
SBUF partition axis; each core takes 16 output rows of all 128 planes.

The problem is HBM-bound on reading K (210 MB f32). The harness gate is
rel_l2 < 2e-2, so K, x and the output travel as bf16 (~2e-3 error), halving
traffic to ~15 MB/core ~= 42 us at the ~358 GB/s per-NC HBM limit.

bf16 also doubles DVE throughput (2x_1P mode), but that mode needs step-1
4B-aligned streams, and the 5 horizontal taps read x at column offsets
0..4 (alternating 2-byte alignment). Fix: keep 5 column-shifted copies of
the padded x band in SBUF (planes 0/1 from host, planes 2/3/4 derived by
the otherwise-idle ScalarE with aligned +2/+4 shifts). A product op then
covers one vertical tap i x all 5 j in a single 3-free-dim AP at 2x.

Reduction of the 25 bf16 product segments runs on the TensorEngine as
1-pass identity matmuls accumulating into one PSUM bank (f32 adds).
ScalarE evacuates PSUM -> SBUF with an f32->bf16 cast; stores are bf16.

Per-core budget: DMA 15.0 MB ~= 42 us (wall), DVE ~31 us, PE ~17-22 us,
ACT ~6 us. Chunks of [4,4,4,3,1] rows keep K loads >=1 MB (near line
rate) and shrink the compute tail after the last K byte lands.
"""

import sys

import numpy as np

sys.path.insert(0, "/opt/trn_rl_repo")

import ml_dtypes

import concourse.bacc as bacc
import concourse.bass as bass
import concourse.tile as tile
from concourse import mybir
from concourse.ap import AP
from concourse.bass_utils import run_bass_kernel_spmd

N_CORES = 8
B, C, H, W, KS = 4, 32, 128, 128, 5
NPLANES = B * C          # 128 -> partition axis
NTAPS = KS * KS          # 25
ROWS_PER_CORE = H // N_CORES   # 16
CHUNK_ROWS = [4, 4, 4, 3, 1]
CHUNK_STARTS = [0, 4, 8, 12, 15]
NCHUNK = len(CHUNK_ROWS)
RMAX = max(CHUNK_ROWS)
FDW = RMAX * W                             # max output elems per chunk-partition
XW = W + KS - 1                            # 132 padded row width
XROWS = ROWS_PER_CORE + KS - 1             # 20 rows incl halo
XPLANE = XROWS * XW                        # 2640 elems per shifted x copy
F32 = mybir.dt.float32
BF16 = mybir.dt.bfloat16
BFNP = ml_dtypes.bfloat16

_compiled = None


def _build_program():
    nc = bacc.Bacc(
        "TRN2",
        target_bir_lowering=False,
        debug=False,
        enable_asserts=False,
        num_devices=N_CORES,
    )
    # Host pre-arranges k as [plane][chunk][tap][h][w] so each chunk load is
    # one contiguous per-partition run (few DMA descriptors, near line rate).
    xd = nc.declare_dram_parameter("x", [NPLANES, XPLANE], BF16, isOutput=False)
    kd = nc.declare_dram_parameter(
        "k", [NPLANES, NTAPS * ROWS_PER_CORE * W], BF16, isOutput=False
    )
    od = nc.declare_dram_parameter("o", [NPLANES, ROWS_PER_CORE * W], BF16, isOutput=True)
    ed = nc.declare_dram_parameter("eye", [NPLANES, NPLANES], BF16, isOutput=False)

    with tile.TileContext(nc) as tc:
        with (
            tc.tile_pool(name="xpool", bufs=1) as xpool,
            tc.tile_pool(name="epool", bufs=1) as epool,
            tc.tile_pool(name="kpool", bufs=3) as kpool,
            tc.tile_pool(name="ppool", bufs=2) as ppool,
            tc.tile_pool(name="spool", bufs=2, space="PSUM") as spool,
            tc.tile_pool(name="opool", bufs=2) as opool,
        ):
            # The tiny eye load (256 B/partition, descriptor-dominated) rides
            # the otherwise-empty ACT ring; x goes FIRST on the sync ring so
            # it is never starved behind the 13 MB K stream (measured 4.5:1).
            et = epool.tile([NPLANES, NPLANES], BF16)
            nc.scalar.dma_start(out=et[:], in_=ed.ap())

            # 5 column-shifted copies of the padded x band: plane j holds
            # xpad[r][c+j]. Planes 0/1 come from HBM (the odd shift cannot be
            # derived on-device at full speed: a +1 source is 2B-misaligned);
            # planes 2/3/4 are +2/+2/+4 shifted flat copies on DVE (aligned
            # even-length 1-D bf16 copies -> 4x mode, ~0.75us each). Copy
            # lengths are trimmed 2-4 elems so each gates on only one x DMA;
            # the missing tail cols (>=128) are never read by the products.
            xt = xpool.tile([NPLANES, 5 * XPLANE], BF16)
            nc.sync.dma_start(out=xt[:, 0:XPLANE], in_=xd.ap()[:, 0:XPLANE])
            # Planes 1-4 are +1..+4 column-shifted flat copies of plane 0,
            # made by the otherwise-idle ScalarE (1 elem/cycle regardless of
            # alignment). Copy lengths are trimmed so nothing reads past the
            # band; the missing tail cols (>=128) are never read. Plane 1
            # gates chunk 0's odd-parity products (~12us); planes 2-4 only
            # gate chunk 1 (~21us).
            for sh in (1, 2, 3, 4):
                nc.scalar.copy(
                    xt[:, sh * XPLANE : (sh + 1) * XPLANE - sh], xt[:, sh:XPLANE]
                )
            xt_ap = xt[:]
            xt_pdim = xt_ap.ap[0]  # (partition step, 128)

            for ch in range(NCHUNK):
                h0 = CHUNK_STARTS[ch]
                rows = CHUNK_ROWS[ch]
                rw = rows * W
                kt = kpool.tile([NPLANES, NTAPS * FDW], BF16, tag="kt")
                base = NTAPS * W * h0
                seg = KS * rw
                # Chunk 0 loads per i-group (5 taps), interleaved with the x
                # planes, so the first products start as early as possible;
                # later chunks use two sub-loads (products for taps 0-9 gate
                # on the first half only). The 1-row tail chunk loads whole.
                # Chunk 0: per-i-group loads (taps host-reordered [0,2,4,1,3]
                # within each i), with i=0 further split 3+2 so the first
                # even-parity product gates on just 0.39 MB. Later chunks use
                # a 10/15-tap split; products for taps 0-9 gate on the first
                # half only. Too many small DMAs serialize on trigger
                # sem-lane reuse, so granularity stays coarse mid-stream.
                if ch == 0:
                    nc.sync.dma_start(
                        out=kt[:, 0 : 3 * rw], in_=kd.ap()[:, base : base + 3 * rw]
                    )
                    nc.sync.dma_start(
                        out=kt[:, 3 * rw : seg],
                        in_=kd.ap()[:, base + 3 * rw : base + seg],
                    )
                    for i in range(1, KS):
                        nc.sync.dma_start(
                            out=kt[:, i * seg : (i + 1) * seg],
                            in_=kd.ap()[:, base + i * seg : base + (i + 1) * seg],
                        )
                else:
                    nc.sync.dma_start(
                        out=kt[:, 0 : 10 * rw],
                        in_=kd.ap()[:, base : base + 10 * rw],
                    )
                    nc.sync.dma_start(
                        out=kt[:, 10 * rw : NTAPS * rw],
                        in_=kd.ap()[:, base + 10 * rw : base + NTAPS * rw],
                    )

                pt = ppool.tile([NPLANES, NTAPS * FDW], BF16, tag="pt")
                st = spool.tile([NPLANES, FDW], F32, tag="st")
                for i in range(KS):
                    if ch == 0:
                        # Parity-split products: even j from plane 0, odd j
                        # from plane 1, j-stride 2 elems (4B) keeps 2x_1P
                        # alignment without waiting for the shifted copies.
                        kt_ap = kt[:]
                        pt_ap = pt[:]
                        for par, nj in ((0, 3), (1, 2)):
                            p0 = i * KS + (0 if par == 0 else 3)
                            k_view = AP(
                                kt_ap.tensor,
                                kt_ap.offset + p0 * rw,
                                [kt_ap.ap[0], (rw, nj), (W, rows), (1, W)],
                            )
                            p_view = AP(
                                pt_ap.tensor,
                                pt_ap.offset + p0 * rw,
                                [pt_ap.ap[0], (rw, nj), (W, rows), (1, W)],
                            )
                            x_view = AP(
                                xt_ap.tensor,
                                xt_ap.offset + par * XPLANE + (h0 + i) * XW,
                                [xt_pdim, (2, nj), (XW, rows), (1, W)],
                            )
                            nc.vector.tensor_mul(p_view, k_view, x_view)
                            for q in range(nj):
                                pos = p0 + q
                                nc.tensor.matmul(
                                    st[:, 0:rw],
                                    et[:],
                                    pt[:, pos * rw : (pos + 1) * rw],
                                    start=(pos == 0),
                                    stop=(pos == NTAPS - 1),
                                )
                        continue
                    # One DVE op per vertical tap i covers the 5 horizontal
                    # taps j as the shifted-plane axis (stride XPLANE): 3 free
                    # dims, all strides even, rows step-1 -> bf16 2x_1P mode.
                    # TensorE folds each i-group's segments into PSUM as soon
                    # as the product lands.
                    k_view = kt[:, i * seg : (i + 1) * seg].rearrange(
                        "p (j h w) -> p j h w", j=KS, h=rows, w=W
                    )
                    p_view = pt[:, i * seg : (i + 1) * seg].rearrange(
                        "p (j h w) -> p j h w", j=KS, h=rows, w=W
                    )
                    x_view = AP(
                        xt_ap.tensor,
                        xt_ap.offset + (h0 + i) * XW,
                        [xt_pdim, (XPLANE, KS), (XW, rows), (1, W)],
                    )
                    nc.vector.tensor_mul(p_view, k_view, x_view)
                    for j in range(KS):
                        t = i * KS + j
                        nc.tensor.matmul(
                            st[:, 0:rw],
                            et[:],
                            pt[:, t * rw : (t + 1) * rw],
                            start=(t == 0),
                            stop=(t == NTAPS - 1),
                        )

                # ScalarE: evacuate PSUM -> SBUF with f32->bf16 cast, store.
                ot = opool.tile([NPLANES, FDW], BF16, tag="ot")
                nc.scalar.copy(ot[:, 0:rw], st[:, 0:rw])
                nc.scalar.dma_start(
                    out=od.ap()[:, h0 * W : h0 * W + rw], in_=ot[:, 0:rw]
                )

    nc.compile()
    return nc


def _get_program():
    global _compiled
    if _compiled is None:
        _compiled = _build_program()
    return _compiled


def _shard_inputs(input: np.ndarray, kernel: np.ndarray):
    x = np.ascontiguousarray(input, dtype=np.float32).reshape(NPLANES, H, W)
    # Edge padding: 2 each side for the conv, +1 extra right col so the
    # odd-shifted copy can take a full 132-wide slice.
    xp = np.pad(x, ((0, 0), (2, 2), (2, 3)), mode="edge").astype(BFNP)  # [128,132,133]
    k = np.ascontiguousarray(kernel, dtype=np.float32).reshape(
        NPLANES, NTAPS, H, W
    )
    eye = np.eye(NPLANES, dtype=BFNP)
    in_maps = []
    for c in range(N_CORES):
        r0 = c * ROWS_PER_CORE
        xb = xp[:, r0 : r0 + XROWS, :]  # [128, 20, 133]
        x2 = np.ascontiguousarray(xb[:, :, 0:XW]).reshape(NPLANES, XPLANE)
        ks = k[:, :, r0 : r0 + ROWS_PER_CORE, :]
        # Chunk 0's taps are reordered [0,2,4,1,3] within each i-group so the
        # even-parity (plane-0) products read contiguous segments and the
        # first product gates on a minimal leading load.
        perm0 = [i * KS + j for i in range(KS) for j in (0, 2, 4, 1, 3)]
        blocks = [
            ks[:, perm0 if ci == 0 else slice(None), s : s + n, :].reshape(
                NPLANES, NTAPS * n * W
            )
            for ci, (s, n) in enumerate(zip(CHUNK_STARTS, CHUNK_ROWS))
        ]
        kc = np.ascontiguousarray(np.concatenate(blocks, axis=1)).astype(BFNP)
        in_maps.append(
            {
                "x": np.ascontiguousarray(x2),
                "k": kc,
                "eye": eye,
            }
        )
    return in_maps


last_results = None  # BassKernelResults of the most recent run (for profiling)


def kernel(input: np.ndarray, kernel: np.ndarray, _trace: bool = False):
    global last_results
    nc = _get_program()
    in_maps = _shard_inputs(input, kernel)
    res = run_bass_kernel_spmd(nc, in_maps, list(range(N_CORES)), trace=_trace)
    last_results = res
    out = np.empty((NPLANES, H, W), dtype=np.float32)
    for c in range(N_CORES):
        out[:, c * ROWS_PER_CORE : (c + 1) * ROWS_PER_CORE, :] = (
            np.asarray(res.results[c]["o"])
            .astype(np.float32)
            .reshape(NPLANES, ROWS_PER_CORE, W)
        )
    return out.reshape(B, C, H, W)


if __name__ == "__main__":
    rng = np.random.default_rng(0)
    inp = rng.standard_normal((B, C, H, W), dtype=np.float32)
    kern = rng.standard_normal((B, C * NTAPS, H, W), dtype=np.float32)
    out = kernel(inp, kern)
    print("ran ok", out.shape, out.dtype)
